# revision 1
# baseline (speedup 1.0000x reference)
"""DeepChebNet (3-layer ChebConv K=3 + MLP head) on 8 Trainium2 NeuronCores.

Strategy (1D node partition per the sharding hint):
  - 50000 nodes padded to 51200, split into two 25600-row half-tables
    (int16 gather index limit). Each core owns 3200 nodes of each half
    (6400 total = 50 x 128-node blocks: 25 "lo" + 25 "hi").
  - Edges are assigned to the core/block owning their dst and grouped by
    the src half; each (block, half) group is padded to 128-edge tiles.
  - Each propagate: per block, batched SWDGE dma_gather calls (<=1024
    rows/call, 4 queues, ~2.5 ns/row) fetch x[src] rows (fp16); DVE
    builds a selection matrix S (S[e, j] = norm_e * (dst_local_e == j))
    via an iota compare; PE accumulates G^T @ S into PSUM, producing the
    feature-major propagated block.
  - Chebyshev recurrence and W-matmuls run feature-major in fp16 with
    f32 PSUM accumulation; bias+ReLU fused on the scalar engine.
  - Updated node half-tables (fp16 node-major) are rebuilt via PE
    transpose and AllGather'd across the 8 cores between propagates
    (halo exchange).
  - MLP head (BN folded into W1/b1 on host) + sigmoid on-chip.
"""
import numpy as np

import concourse.bacc as bacc
import concourse.bass as bass
import concourse.mybir as mybir
import concourse.tile as tile
from concourse.bass_utils import run_bass_kernel_spmd
from concourse.masks import make_identity

# problem constants (hardcoded per harness contract)
N_NODES = 50000
N_EDGES = 800000
D = 128
K = 3
BN_EPS = 1e-5

N_CORES = 8
P = 128
N_PAD = 51200
HALF = 25600            # rows per half-table (< 32768: int16-indexable)
HSLAB = 3200            # per-core nodes per half
BLK_NODES = 6400        # per-core nodes
N_BLOCKS = 50           # per-core 128-node blocks (25 lo + 25 hi)
N_HB = 25               # blocks per half per core
T_CAP = 8               # max tiles (128 rows) per dma_gather call
N_QUEUES = 4

F16 = mybir.dt.float16
F32 = mybir.dt.float32
npf16 = np.float16


def _owner_block(n):
    """global node id -> (core, block 0..49) under the lo/hi layout."""
    lo = n < HALF
    core = np.where(lo, n // HSLAB, (n - HALF) // HSLAB)
    blk = np.where(lo, (n % HSLAB) // P, N_HB + ((n - HALF) % HSLAB) // P)
    return core, blk


def _preprocess(edge_index, edge_weight):
    """Graph partition + per-core edge streams (dst-block / src-half)."""
    src = np.asarray(edge_index[0], dtype=np.int64)
    dst = np.asarray(edge_index[1], dtype=np.int64)
    w = np.asarray(edge_weight, dtype=np.float32)

    deg = np.bincount(src, weights=w.astype(np.float64), minlength=N_NODES)
    deg = deg.astype(np.float32)
    degs = np.sqrt(np.maximum(deg, 1e-38))
    dinv = np.where(deg > 0, 1.0 / degs, 0.0).astype(np.float32)
    norm = (-dinv[src] * w * dinv[dst]).astype(np.float32)

    core, blk = _owner_block(dst)
    half = (src >= HALF).astype(np.int64)
    key = (core * N_BLOCKS + blk) * 2 + half
    order = np.argsort(key, kind="stable")
    src_s, dst_s, norm_s, key_s = src[order], dst[order], norm[order], key[order]

    n_groups = N_CORES * N_BLOCKS * 2    # 800 (core, block, half) groups
    bounds = np.searchsorted(key_s, np.arange(n_groups + 1))
    counts = bounds[1:] - bounds[:-1]
    t_half = max(1, int(np.max((counts + P - 1) // P)))  # tiles per group

    idx_all, smat_all = [], []
    n_tiles = N_BLOCKS * 2 * t_half
    for c in range(N_CORES):
        gslots = t_half * P
        n_slots = N_BLOCKS * 2 * gslots
        e_src = np.zeros(n_slots, dtype=np.int16)
        e_keep0 = np.zeros(n_slots, dtype=bool)
        e_dstl = np.zeros(n_slots, dtype=np.int64)
        e_norm = np.zeros(n_slots, dtype=np.float32)
        e_live = np.zeros(n_slots, dtype=bool)
        for b in range(N_BLOCKS):
            for h in range(2):
                gidx = (c * N_BLOCKS + b) * 2 + h
                lo, hi = bounds[gidx], bounds[gidx + 1]
                n = hi - lo
                base = (b * 2 + h) * gslots
                e_src[base:base + n] = (src_s[lo:hi] - h * HALF).astype(np.int16)
                e_dstl[base:base + n] = dst_s[lo:hi] % P
                e_norm[base:base + n] = norm_s[lo:hi]
                e_live[base:base + n] = True
                # keep >=1 full tile of valid (0) idxs per call; -1 only for
                # whole trailing empty tiles (ucode trims; <16 descs hangs)
                keep = max(((n + P - 1) // P) * P, P)
                e_keep0[base + n:base + keep] = True
        e_src[e_keep0] = 0
        # int16 idx stream: per (b, h) group, flat i -> (row i%16, col i//16),
        # replicated across the 8 groups of 16 partitions.
        n_grp = N_BLOCKS * 2
        per_grp = np.transpose(
            e_src.reshape(n_grp, gslots // 16, 16), (0, 2, 1))
        arr = np.concatenate([per_grp[i] for i in range(n_grp)], axis=1)
        idx16 = np.zeros((P, n_grp * (gslots // 16)), dtype=np.int16)
        for gs in range(8):
            idx16[gs * 16:(gs + 1) * 16, :] = arr
        idx_all.append(np.ascontiguousarray(idx16))
        # precomputed selection matrices: smat[p, gt*P + j] =
        #   norm_e if (tile gt, lane p) holds edge e with dst_local j
        slot = np.nonzero(e_live)[0]
        gt, lane = slot // P, slot % P
        s_all = np.zeros(n_tiles * P * P, dtype=np.float16)
        s_all[(gt * P + lane) * P + e_dstl[slot]] = e_norm[slot]
        smat = np.ascontiguousarray(
            s_all.reshape(n_tiles, P, P).transpose(1, 0, 2).reshape(P, -1))
        smat_all.append(smat)
    return t_half, idx_all, smat_all


def _build_program(t_half, b2_val):
    """Build the SPMD Bass program (identical across cores)."""
    nc = bacc.Bacc("TRN2", target_bir_lowering=False, debug=False,
                   num_devices=N_CORES, num_swdge_queues=N_QUEUES)

    t_blk = 2 * t_half           # tiles per block
    n_tiles = N_BLOCKS * t_blk
    sw = t_blk * P               # S / G width per block
    gcols = t_half * P // 16     # idx columns per (block, half) group

    # ---- I/O -----------------------------------------------------------
    xA = nc.dram_tensor("xA", [HALF, D], F16, kind="ExternalInput")
    xB = nc.dram_tensor("xB", [HALF, D], F16, kind="ExternalInput")
    x0fm = nc.dram_tensor("x0fm", [P, BLK_NODES], F16, kind="ExternalInput")
    idx_d = nc.dram_tensor("idx", [P, 2 * N_BLOCKS * gcols], mybir.dt.int16,
                           kind="ExternalInput")
    smat_d = nc.dram_tensor("smat", [P, n_tiles * P], F16,
                            kind="ExternalInput")
    wts_d = nc.dram_tensor("wts", [P, 9 * D + D + 1], F16, kind="ExternalInput")
    bias_d = nc.dram_tensor("bias", [P, 4], F32, kind="ExternalInput")
    y_d = nc.dram_tensor("y", [1, BLK_NODES], F32, kind="ExternalOutput")

    tabsA = [nc.dram_tensor(f"tabA{i}", [HALF, D], F16, addr_space="Shared")
             for i in range(5)]
    tabsB = [nc.dram_tensor(f"tabB{i}", [HALF, D], F16, addr_space="Shared")
             for i in range(5)]
    rg = [list(range(N_CORES))]

    with tile.TileContext(nc) as tc:
        with (
            tc.tile_pool(name="const", bufs=1) as constp,
            tc.tile_pool(name="big", bufs=1) as bigp,
            tc.tile_pool(name="gat", bufs=12) as gatp,
            tc.tile_pool(name="sel", bufs=8) as selp,
            tc.tile_pool(name="nm", bufs=3) as nmp,
            tc.tile_pool(name="ps", bufs=4, space="PSUM") as psp,
            tc.tile_pool(name="pst", bufs=2, space="PSUM") as pstp,
            tc.tile_pool(name="pso", bufs=2, space="PSUM") as psop,
            tc.tile_pool(name="dram", bufs=1, space="DRAM") as dramp,
        ):
            # ---- load constants -----------------------------------------
            idx_t = constp.tile([P, 2 * N_BLOCKS * gcols], mybir.dt.int16)
            wts_t = constp.tile([P, 9 * D + D + 1], F16)
            bias_t = constp.tile([P, 4], F32)
            ident = constp.tile([P, P], F16)
            nc.sync.dma_start(idx_t[:], idx_d[:])
            nc.sync.dma_start(wts_t[:], wts_d[:])
            nc.sync.dma_start(bias_t[:], bias_d[:])
            make_identity(nc, ident[:])

            def wslice(i):  # i-th [P, D] weight block (lhsT layout [fi, fo])
                return wts_t[:, i * D:(i + 1) * D]

            w2_ap = wts_t[:, 10 * D:10 * D + 1]

            # ---- big feature-major activations [P, 6400] f16 ------------
            tA = bigp.tile([P, BLK_NODES], F16, tag="tA")
            tB = bigp.tile([P, BLK_NODES], F16, tag="tB")
            tC = bigp.tile([P, BLK_NODES], F16, tag="tC")
            tD = bigp.tile([P, BLK_NODES], F16, tag="tD")
            nc.sync.dma_start(tA[:], x0fm[:])

            bncA = [dramp.tile([HSLAB, D], F16, tag=f"bncA{i}", name=f"bncA{i}")
                    for i in range(5)]
            bncB = [dramp.tile([HSLAB, D], F16, tag=f"bncB{i}", name=f"bncB{i}")
                    for i in range(5)]

            qctr = [0]

            def propagate(srcA, srcB, out_fm, tx0_fm=None, table=None):
                """out_fm[:, blk] = (A_hat @ table)^T per block; if tx0_fm is
                given, out = 2*prop - tx0 (second Chebyshev step). If table is
                (blo, bhi, tabA, tabB), also emit the node-major table and
                fire the AllGathers as soon as each half completes."""
                for b in range(N_BLOCKS):
                    g = gatp.tile([P, sw], F16, tag="g")
                    for h, src_tab in ((0, srcA), (1, srcB)):
                        grp = b * 2 + h
                        nc.gpsimd.dma_gather(
                            out_ap=g[:, h * t_half * P:(h + 1) * t_half * P]
                                .rearrange("p (n d) -> p n d", d=D),
                            in_ap=src_tab[:],
                            idxs_ap=idx_t[:, grp * gcols:(grp + 1) * gcols],
                            num_idxs=t_half * P,
                            num_idxs_reg=t_half * P,
                            elem_size=D,
                            queue_num=qctr[0] % N_QUEUES,
                            single_packet=False,
                        )
                        qctr[0] += 1
                    s = selp.tile([P, sw], F16, tag="s")
                    nc.scalar.dma_start(
                        s[:], smat_d[:, b * sw:(b + 1) * sw])
                    ps = psp.tile([P, P], F32, tag="ps", space="PSUM")
                    for t in range(t_blk):
                        nc.tensor.matmul(
                            out=ps[:],
                            lhsT=g[:, t * P:(t + 1) * P],
                            rhs=s[:, t * P:(t + 1) * P],
                            start=(t == 0), stop=(t == t_blk - 1),
                        )
                    osl = out_fm[:, b * P:(b + 1) * P]
                    if tx0_fm is None:
                        nc.vector.tensor_copy(out=osl, in_=ps[:])
                    else:
                        nc.vector.scalar_tensor_tensor(
                            out=osl, in0=ps[:], scalar=2.0,
                            in1=tx0_fm[:, b * P:(b + 1) * P],
                            op0=mybir.AluOpType.mult,
                            op1=mybir.AluOpType.subtract)
                    if table is not None:
                        blo, bhi, tabA, tabB = table
                        pt = pstp.tile([P, P], F16, tag="pt", space="PSUM")
                        nc.tensor.transpose(pt[:], osl, ident[:])
                        nm = nmp.tile([P, P], F16, tag="nm")
                        nc.vector.tensor_copy(out=nm[:], in_=pt[:])
                        if b < N_HB:
                            nc.sync.dma_start(blo[b * P:(b + 1) * P, :], nm[:])
                        else:
                            bb = b - N_HB
                            nc.sync.dma_start(bhi[bb * P:(bb + 1) * P, :], nm[:])
                        if b == N_HB - 1:
                            nc.gpsimd.collective_compute(
                                "AllGather", mybir.AluOpType.bypass,
                                replica_groups=rg, ins=[blo[:]], outs=[tabA[:]])
                        elif b == N_BLOCKS - 1:
                            nc.gpsimd.collective_compute(
                                "AllGather", mybir.AluOpType.bypass,
                                replica_groups=rg, ins=[bhi[:]], outs=[tabB[:]])

            def build_table(fm, blo, bhi, tabA, tabB):
                """node-major fp16 half-tables from feature-major activations.

                AllGather of the lo half fires as soon as blocks 0..24 are
                written so it overlaps the hi-half transposes and the CC
                queue pipelines the two collectives."""
                for b in range(N_BLOCKS):
                    pt = pstp.tile([P, P], F16, tag="pt", space="PSUM")
                    nc.tensor.transpose(pt[:], fm[:, b * P:(b + 1) * P], ident[:])
                    nm = nmp.tile([P, P], F16, tag="nm")
                    nc.vector.tensor_copy(out=nm[:], in_=pt[:])
                    if b < N_HB:
                        nc.sync.dma_start(blo[b * P:(b + 1) * P, :], nm[:])
                    else:
                        bb = b - N_HB
                        nc.sync.dma_start(bhi[bb * P:(bb + 1) * P, :], nm[:])
                    if b == N_HB - 1:
                        nc.gpsimd.collective_compute(
                            "AllGather", mybir.AluOpType.bypass,
                            replica_groups=rg, ins=[blo[:]], outs=[tabA[:]])
                nc.gpsimd.collective_compute(
                    "AllGather", mybir.AluOpType.bypass, replica_groups=rg,
                    ins=[bhi[:]], outs=[tabB[:]])

            def cheb_out(tx0, tx1, tx2, wbase, bias_col, relu, out_fm):
                """out = Tx0@W0 + Tx1@W1 + Tx2@W2 + b (+ReLU), feature-major."""
                for g0 in range(0, BLK_NODES, 512):
                    gw = min(512, BLK_NODES - g0)
                    po = psop.tile([P, 512], F32, tag="po", space="PSUM")
                    for k, txk in enumerate((tx0, tx1, tx2)):
                        nc.tensor.matmul(
                            out=po[:, :gw], lhsT=wslice(wbase + k),
                            rhs=txk[:, g0:g0 + gw],
                            start=(k == 0), stop=(k == 2))
                    if relu:
                        nc.scalar.activation(
                            out_fm[:, g0:g0 + gw], po[:, :gw],
                            mybir.ActivationFunctionType.Relu,
                            bias=bias_t[:, bias_col:bias_col + 1], scale=1.0)
                    else:
                        nc.vector.tensor_tensor(
                            out=out_fm[:, g0:g0 + gw], in0=po[:, :gw],
                            in1=bias_t[:, bias_col:bias_col + 1]
                                .broadcast_to([P, gw]),
                            op=mybir.AluOpType.add)

            # ================= layer 1 =================
            propagate(xA, xB, tB,
                      table=(bncA[0], bncB[0], tabsA[0], tabsB[0]))  # Tx1
            propagate(tabsA[0], tabsB[0], tC, tx0_fm=tA)  # Tx2
            cheb_out(tA, tB, tC, 0, 0, True, tD)          # h1
            build_table(tD, bncA[1], bncB[1], tabsA[1], tabsB[1])

            # ================= layer 2 =================
            propagate(tabsA[1], tabsB[1], tB,
                      table=(bncA[2], bncB[2], tabsA[2], tabsB[2]))
            propagate(tabsA[2], tabsB[2], tC, tx0_fm=tD)
            cheb_out(tD, tB, tC, 3, 1, True, tA)          # h2
            build_table(tA, bncA[3], bncB[3], tabsA[3], tabsB[3])

            # ================= layer 3 =================
            propagate(tabsA[3], tabsB[3], tB,
                      table=(bncA[4], bncB[4], tabsA[4], tabsB[4]))
            propagate(tabsA[4], tabsB[4], tC, tx0_fm=tA)
            cheb_out(tA, tB, tC, 6, 2, False, tD)         # h3

            # ================= MLP head =================
            for g0 in range(0, BLK_NODES, 512):
                gw = min(512, BLK_NODES - g0)
                pm = psop.tile([P, 512], F32, tag="po", space="PSUM")
                nc.tensor.matmul(out=pm[:, :gw], lhsT=wslice(9),
                                 rhs=tD[:, g0:g0 + gw], start=True, stop=True)
                h4 = nmp.tile([P, 512], F16, tag="h4")
                nc.scalar.activation(h4[:, :gw], pm[:, :gw],
                                     mybir.ActivationFunctionType.Relu,
                                     bias=bias_t[:, 3:4], scale=1.0)
                p2 = psop.tile([1, 512], F32, tag="po", space="PSUM")
                nc.tensor.matmul(out=p2[:, :gw], lhsT=w2_ap,
                                 rhs=h4[:, :gw], start=True, stop=True)
                yo = nmp.tile([1, 512], F32, tag="yo")
                nc.scalar.activation(yo[:, :gw], p2[:, :gw],
                                     mybir.ActivationFunctionType.Sigmoid,
                                     bias=b2_val, scale=1.0)
                nc.sync.dma_start(y_d[:, g0:g0 + gw], yo[:1, :gw])

    nc.finalize()
    return nc


_CACHE = {}


def kernel(x, edge_index, edge_weight, W_in, b_in, W_hid, b_hid, W_out, b_out,
           mlp_w1, mlp_b1, bn_gamma, bn_beta, bn_mean, bn_var, mlp_w2, mlp_b2,
           _trace=False):
    x = np.asarray(x, dtype=np.float32)
    t_half, idx_all, smat_all = _preprocess(
        np.asarray(edge_index), np.asarray(edge_weight))

    b2_val = float(np.asarray(mlp_b2, np.float32).reshape(-1)[0])
    cache_key = (t_half, b2_val)
    if cache_key in _CACHE:
        nc = _CACHE[cache_key]
    else:
        nc = _build_program(t_half, b2_val)
        _CACHE[cache_key] = nc

    # ---- host-side tensor prep ----------------------------------------
    xpad = np.zeros((N_PAD, D), dtype=np.float32)
    xpad[:N_NODES] = x
    x16_np = xpad.astype(npf16)

    # BN folding: y = s*(h@W1 + b1) + t -> W1' = W1*s, b1' = b1*s + t
    s = (np.asarray(bn_gamma, np.float32)
         / np.sqrt(np.asarray(bn_var, np.float32) + BN_EPS))
    t_ = np.asarray(bn_beta, np.float32) - np.asarray(bn_mean, np.float32) * s
    w1p = np.asarray(mlp_w1, np.float32) * s[None, :]
    b1p = np.asarray(mlp_b1, np.float32) * s + t_

    wts = np.zeros((P, 9 * D + D + 1), dtype=npf16)
    for i, W in enumerate((W_in, W_hid, W_out)):
        W = np.asarray(W, np.float32)
        for k in range(K):
            wts[:, (i * K + k) * D:(i * K + k + 1) * D] = W[k].astype(npf16)
    wts[:, 9 * D:10 * D] = w1p.astype(npf16)
    wts[:, 10 * D:10 * D + 1] = np.asarray(mlp_w2, np.float32).astype(npf16)

    biases = np.zeros((P, 4), dtype=np.float32)
    biases[:, 0] = np.asarray(b_in, np.float32)
    biases[:, 1] = np.asarray(b_hid, np.float32)
    biases[:, 2] = np.asarray(b_out, np.float32)
    biases[:, 3] = b1p

    in_maps = []
    for c in range(N_CORES):
        own = np.concatenate([
            x16_np[c * HSLAB:(c + 1) * HSLAB],
            x16_np[HALF + c * HSLAB:HALF + (c + 1) * HSLAB],
        ], axis=0)  # [6400, 128]
        in_maps.append({
            "xA": x16_np[:HALF],
            "xB": x16_np[HALF:],
            "x0fm": np.ascontiguousarray(own.T),
            "idx": idx_all[c],
            "smat": smat_all[c],
            "wts": wts,
            "bias": biases,
        })

    res = run_bass_kernel_spmd(nc, in_maps, list(range(N_CORES)), trace=_trace)
    y_full = np.zeros(N_PAD, dtype=np.float32)
    for c in range(N_CORES):
        yc = res.results[c]["y"][0]
        y_full[c * HSLAB:(c + 1) * HSLAB] = yc[:HSLAB]
        y_full[HALF + c * HSLAB:HALF + (c + 1) * HSLAB] = yc[HSLAB:]
    out = y_full[:N_NODES, None].astype(np.float32)
    if _trace:
        kernel._last_results = res
    return out

